# revision 10
# baseline (speedup 1.0000x reference)
"""Trainium2 Bass kernel for nn_CombinedLoss (rec + ident + attention-BCE).

Strategy
--------
The 256 MB correspondence_matrices BCE dominates (memory-bound).  Key identity:
gt_corr is nonzero only on the 5 diagonals |i-j|<=2 of each [N,N] matrix, so

    bce = w*(relu(x) - x*g + log1p(exp(-|x|)))   with w = 1+2g
        = softplus(x)                            off-band (~all elements)
        = softplus(x) + [2g*softplus(x) - (g+2g^2)*x]   on the band

Main stream (per core, 8 matrices, host-converted to bf16 = 16 MB DMA):
softplus sums are split across two engines so neither is the bottleneck:
  - K_SIG chunks: ScalarE Sigmoid pass, then DVE computes
    softplus = relu(x) + p(m), m = min(s, 1-s), p = deg-2 minimax poly of
    -ln(1-m) on [0, 0.5] (exact identity softplus = relu - ln(1-m)); the
    relu and poly sums come from fused accum_out ops (5 DVE ops/chunk).
  - remaining chunks: ScalarE Exp then Ln(bias=1) with accum_out; all Exps
    grouped before all Lns so each ACT table loads once (3 loads total).
Band correction: diagonals are extracted host-side into a dense [128,320]
f32 tile; pair visibility is folded into host-precomputed weights.

rec + ident losses are tiny (~3.5 MB) and sharded uniformly: each core takes
1/8 of the reconstruction points and 2 of the 16 (view,batch) identity pairs.
Each core writes a [128,24] f32 partial; the host only combines partials.
"""

import dataclasses
import os

import ml_dtypes
import numpy as np

import concourse.bacc as bacc
import concourse.bass as bass
import concourse.mybir as mybir
from concourse.bass_utils import run_bass_kernel_spmd
from concourse.tile import TileContext

F32 = mybir.dt.float32
BF16 = mybir.dt.bfloat16
I32 = mybir.dt.int32
AF = mybir.ActivationFunctionType
OP = mybir.AluOpType
AX = mybir.AxisListType

N = 1024
V = 4
B = 4
F_FRAMES = 16
NCORES = 8
MAT_PER_CORE = 8          # V*V*B / 8
MATSZ = N * N             # elements per matrix
NCHUNKS = 16
CHUNK = MAT_PER_CORE * MATSZ // NCHUNKS      # 524288 elems
CHUNK_F = CHUNK // 128                       # free dim per partition

K_SIG = int(os.environ.get("KERNEL_KSIG", "10"))

# deg-2 minimax fit of -ln(1-m) ~ m*(PC1 + PC2*m) on m in [0, 0.5]
PC1 = 0.92217051
PC2 = 0.91185078

# rec shard: 1/8 of B*F*N = 65536 points -> 8192 points = [128, 64, 3]
REC_PTS = 8192

# final_acc column layout
C_ATT_MAIN = 0
C_ATT_C1 = 1
C_REC_SE = 2
C_REC_NUM = 3
C_REC_MN = 4     # 4..6
C_REC_MX = 7     # 7..9
C_ID_ERR = 10    # 10..13 (vb0x, vb0y, vb1x, vb1y)
C_ID_MN = 14     # 14..17
C_ID_MX = 18     # 18..21
C_ATT_C2 = 22
NCOLS = 24

NACC = 32        # acc cols: 0..15 chunk (ln or poly) accums, 16..31 relu accums

_CACHE = {}
LAST_RESULTS = None


def _ap(t, offset, pairs):
    """Custom access pattern on a DRAM tensor handle."""
    return dataclasses.replace(t[:], ap=[list(p) for p in pairs], offset=offset)


def _build_program():
    parts = set(
        os.environ.get("KERNEL_PARTS", "main,band,rec,ident").split(",")
    )
    nc = bacc.Bacc("TRN2", target_bir_lowering=False, debug=False)

    corrm = nc.dram_tensor(
        "corrm", [MAT_PER_CORE * MATSZ], BF16, kind="ExternalInput"
    )
    bnd_d = nc.dram_tensor("bnd", [128, 320], F32, kind="ExternalInput")
    g1_d = nc.dram_tensor("g1", [128, 320], F32, kind="ExternalInput")
    g2_d = nc.dram_tensor("g2", [128, 320], F32, kind="ExternalInput")
    recpred = nc.dram_tensor("recpred", [128, 192], F32, kind="ExternalInput")
    recgt = nc.dram_tensor("recgt", [128, 192], F32, kind="ExternalInput")
    recvis = nc.dram_tensor("recvis", [128, 64], I32, kind="ExternalInput")
    trk = nc.dram_tensor("trk", [2, 128, 256], F32, kind="ExternalInput")
    iprd = nc.dram_tensor("iprd", [2, 128, 384], F32, kind="ExternalInput")
    projbc = nc.dram_tensor("projbc", [128, 24], F32, kind="ExternalInput")
    out_d = nc.dram_tensor("out", [128, NCOLS], F32, kind="ExternalOutput")

    with TileContext(nc) as tc:
        with (
            tc.tile_pool(name="xpool", bufs=3) as xpool,
            tc.tile_pool(name="spool", bufs=2) as spool,
            tc.tile_pool(name="wpool", bufs=1) as wpool,
            tc.tile_pool(name="cpool", bufs=1) as cpool,
        ):
            acc = cpool.tile([128, NACC], F32, tag="acc")
            fin = cpool.tile([128, NCOLS], F32, tag="fin")
            nc.vector.memset(acc[:], 0.0)
            nc.vector.memset(fin[:], 0.0)

            if "band" in parts:
                bt = cpool.tile([128, 320], F32, tag="bt")
                g1t = cpool.tile([128, 320], F32, tag="g1t")
                g2t = cpool.tile([128, 320], F32, tag="g2t")
                nc.gpsimd.dma_start(bt[:], bnd_d[:])
                nc.gpsimd.dma_start(g1t[:], g1_d[:])
                nc.gpsimd.dma_start(g2t[:], g2_d[:])

            # ---------------- main BCE stream ----------------
            # phase S: sigmoid + DVE polynomial chunks
            if "main" in parts:
                for k in range(K_SIG):
                    xt = xpool.tile([128, CHUNK_F], BF16, tag="xt")
                    nc.sync.dma_start(
                        xt[:],
                        _ap(corrm, k * CHUNK, [[CHUNK_F, 128], [1, CHUNK_F]]),
                    )
                    st = spool.tile([128, CHUNK_F], BF16, tag="st")
                    nc.scalar.activation(st[:], xt[:], AF.Sigmoid)
                    # relu(x) with fused sum
                    rt = wpool.tile([128, CHUNK_F], BF16, tag="rt")
                    nc.vector.tensor_scalar(
                        rt[:], xt[:], 0.0, None, OP.max, OP.add,
                        accum_out=acc[:, 16 + k : 17 + k],
                    )
                    # m = min(s, 1-s)
                    nst = wpool.tile([128, CHUNK_F], BF16, tag="nst")
                    nc.vector.tensor_scalar(
                        nst[:], st[:], -1.0, 1.0, OP.mult, OP.add
                    )
                    mt = wpool.tile([128, CHUNK_F], BF16, tag="mt")
                    nc.vector.tensor_tensor(mt[:], st[:], nst[:], OP.min)
                    # p(m) = m*(PC1 + PC2*m), fused sum
                    yt = wpool.tile([128, CHUNK_F], BF16, tag="yt")
                    nc.vector.tensor_scalar(
                        yt[:], mt[:], PC2, PC1, OP.mult, OP.add
                    )
                    wt = wpool.tile([128, CHUNK_F], BF16, tag="wt")
                    nc.vector.scalar_tensor_tensor(
                        wt[:], yt[:], 0.0, mt[:], OP.add, OP.mult,
                        accum_out=acc[:, k : k + 1],
                    )

                # phase E: exp for the remaining chunks (one table load)
                ets = {}
                for k in range(K_SIG, NCHUNKS):
                    xt = xpool.tile([128, CHUNK_F], BF16, tag="xt")
                    nc.sync.dma_start(
                        xt[:],
                        _ap(corrm, k * CHUNK, [[CHUNK_F, 128], [1, CHUNK_F]]),
                    )
                    et = cpool.tile([128, CHUNK_F], BF16, tag=f"et{k}")
                    nc.scalar.activation(et[:], xt[:], AF.Exp)
                    ets[k] = et

            if "band" in parts:
                eb = cpool.tile([128, 320], F32, tag="eb")
                nc.scalar.activation(eb[:], bt[:], AF.Exp)

            # phase L: ln(1+e^x) with fused accum (one table load)
            if "main" in parts:
                for k in range(K_SIG, NCHUNKS):
                    ot = spool.tile([128, CHUNK_F], BF16, tag="ot")
                    nc.scalar.activation(
                        ot[:], ets[k][:], AF.Ln, bias=1.0,
                        accum_out=acc[:, k : k + 1],
                    )

            if "band" in parts:
                spb = cpool.tile([128, 320], F32, tag="spb")
                nc.scalar.activation(spb[:], eb[:], AF.Ln, bias=1.0)
                scr1 = cpool.tile([128, 320], F32, tag="scr1")
                scr2 = cpool.tile([128, 320], F32, tag="scr2")
                nc.vector.tensor_tensor(scr1[:], g1t[:], spb[:], OP.mult)
                nc.vector.tensor_reduce(
                    fin[:, C_ATT_C1 : C_ATT_C1 + 1], scr1[:], axis=AX.X, op=OP.add
                )
                nc.vector.tensor_tensor(scr2[:], g2t[:], bt[:], OP.mult)
                nc.vector.tensor_reduce(
                    fin[:, C_ATT_C2 : C_ATT_C2 + 1], scr2[:], axis=AX.X, op=OP.add
                )

            # ---------------- reconstruction loss partials ----------------
            if "rec" in parts:
                prt = cpool.tile([128, 192], F32, tag="prt")
                grt = cpool.tile([128, 192], F32, tag="grt")
                vrt = cpool.tile([128, 64], I32, tag="vrt")
                nc.gpsimd.dma_start(prt[:], recpred[:])
                nc.gpsimd.dma_start(grt[:], recgt[:])
                nc.gpsimd.dma_start(vrt[:], recvis[:])
                mf = cpool.tile([128, 64], F32, tag="mf")
                nc.vector.tensor_copy(mf[:], vrt[:])
                dd = cpool.tile([128, 192], F32, tag="dd")
                nc.vector.tensor_tensor(dd[:], prt[:], grt[:], OP.subtract)
                d2 = cpool.tile([128, 192], F32, tag="d2")
                nc.vector.tensor_tensor(d2[:], dd[:], dd[:], OP.mult)
                se3 = cpool.tile([128, 64], F32, tag="se3")
                nc.vector.tensor_reduce(
                    se3[:], d2[:].rearrange("p (q c) -> p q c", c=3), axis=AX.X, op=OP.add
                )
                se3m = cpool.tile([128, 64], F32, tag="se3m")
                nc.vector.tensor_tensor(se3m[:], se3[:], mf[:], OP.mult)
                nc.vector.tensor_reduce(
                    fin[:, C_REC_SE : C_REC_SE + 1], se3m[:], axis=AX.X, op=OP.add
                )
                nc.vector.tensor_reduce(
                    fin[:, C_REC_NUM : C_REC_NUM + 1], mf[:], axis=AX.X, op=OP.add
                )
                # masked min / max of gt per coordinate
                bmn = cpool.tile([128, 192], F32, tag="bmn")
                bmx = cpool.tile([128, 192], F32, tag="bmx")
                nc.vector.memset(bmn[:], 1e30)
                nc.vector.memset(bmx[:], -1e30)
                for c in range(3):
                    nc.vector.copy_predicated(
                        bmn[:, c : 192 : 3], vrt[:], grt[:, c : 192 : 3]
                    )
                    nc.vector.copy_predicated(
                        bmx[:, c : 192 : 3], vrt[:], grt[:, c : 192 : 3]
                    )
                nc.vector.tensor_reduce(
                    fin[:, C_REC_MN : C_REC_MN + 3],
                    bmn[:].rearrange("p (q c) -> p c q", c=3), axis=AX.X, op=OP.min,
                )
                nc.vector.tensor_reduce(
                    fin[:, C_REC_MX : C_REC_MX + 3],
                    bmx[:].rearrange("p (q c) -> p c q", c=3), axis=AX.X, op=OP.max,
                )

            # ---------------- identity loss partials (2 vb slots) ----------------
            if "ident" in parts:
                psb = cpool.tile([128, 24], F32, tag="psb")
                nc.gpsimd.dma_start(psb[:], projbc[:])

                for i in range(2):
                    tk = cpool.tile([128, 256], F32, tag=f"tk{i}")
                    pd = cpool.tile([128, 384], F32, tag=f"pd{i}")
                    nc.gpsimd.dma_start(tk[:], trk[i])
                    nc.gpsimd.dma_start(pd[:], iprd[i])
                    Xc = pd[:, 0:384:3]
                    Yc = pd[:, 1:384:3]
                    Zc = pd[:, 2:384:3]

                    def cS(col):
                        return psb[:, col : col + 1]

                    base = i * 12

                    def lincomb(row, tag):
                        # P[row,0]*x + P[row,1]*y + P[row,2]*z + P[row,3]
                        t0 = cpool.tile([128, 128], F32, tag=f"{tag}0_{i}")
                        t1 = cpool.tile([128, 128], F32, tag=f"{tag}1_{i}")
                        nc.vector.tensor_scalar(
                            t0[:], Xc, cS(base + row * 4 + 0), cS(base + row * 4 + 3),
                            OP.mult, OP.add,
                        )
                        nc.vector.tensor_scalar(
                            t1[:], Yc, cS(base + row * 4 + 1), None, OP.mult
                        )
                        nc.vector.tensor_tensor(t0[:], t0[:], t1[:], OP.add)
                        nc.vector.tensor_scalar(
                            t1[:], Zc, cS(base + row * 4 + 2), None, OP.mult
                        )
                        nc.vector.tensor_tensor(t0[:], t0[:], t1[:], OP.add)
                        return t0

                    den = lincomb(2, "den")
                    nc.vector.tensor_scalar_add(den[:], den[:], 1e-10)
                    rd = cpool.tile([128, 128], F32, tag=f"rd{i}")
                    nc.vector.reciprocal(rd[:], den[:])
                    nx = lincomb(0, "nx")
                    ny = lincomb(1, "ny")
                    nc.vector.tensor_tensor(nx[:], nx[:], rd[:], OP.mult)
                    nc.vector.tensor_tensor(ny[:], ny[:], rd[:], OP.mult)
                    nc.vector.tensor_tensor(nx[:], nx[:], tk[:, 0:256:2], OP.subtract)
                    nc.vector.tensor_tensor(ny[:], ny[:], tk[:, 1:256:2], OP.subtract)
                    sqx = cpool.tile([128, 128], F32, tag=f"sqx{i}")
                    nc.vector.tensor_tensor(sqx[:], nx[:], nx[:], OP.mult)
                    nc.vector.tensor_reduce(
                        fin[:, C_ID_ERR + 2 * i : C_ID_ERR + 2 * i + 1],
                        sqx[:], axis=AX.X, op=OP.add,
                    )
                    sqy = cpool.tile([128, 128], F32, tag=f"sqy{i}")
                    nc.vector.tensor_tensor(sqy[:], ny[:], ny[:], OP.mult)
                    nc.vector.tensor_reduce(
                        fin[:, C_ID_ERR + 2 * i + 1 : C_ID_ERR + 2 * i + 2],
                        sqy[:], axis=AX.X, op=OP.add,
                    )
                    tkv = tk[:].rearrange("p (q c) -> p c q", c=2)
                    nc.vector.tensor_reduce(
                        fin[:, C_ID_MN + 2 * i : C_ID_MN + 2 * i + 2],
                        tkv, axis=AX.X, op=OP.min,
                    )
                    nc.vector.tensor_reduce(
                        fin[:, C_ID_MX + 2 * i : C_ID_MX + 2 * i + 2],
                        tkv, axis=AX.X, op=OP.max,
                    )

            # ---------------- final reductions + store ----------------
            nc.vector.tensor_reduce(
                fin[:, C_ATT_MAIN : C_ATT_MAIN + 1], acc[:], axis=AX.X, op=OP.add
            )
            nc.sync.dma_start(out_d[:], fin[:])

    nc.compile()
    return nc


def _host_band_tables():
    """Banded weight tables with pair visibility folded in (host-side)."""
    beta = np.array([0.49, 0.7, 1.0, 0.7, 0.49], np.float64)
    r = np.arange(N)
    d_off = np.arange(5) - 2
    col = r[:, None] + d_off[None, :]                  # [1024,5]
    validc = (col >= 0) & (col < N)
    colc = np.clip(col, 0, N - 1)
    # flat index of band element (r, r+d) within one matrix, clipped
    flat_idx = np.clip(r[:, None] * 1025 + np.arange(5)[None, :] - 2, 0, MATSZ - 1)
    return beta, validc, colc, flat_idx


def _to_dev_band(a):
    """[8,1024,5] -> [128, 320] with partition p holding rows 8p..8p+7."""
    return np.ascontiguousarray(
        a.reshape(8, 128, 8, 5).transpose(1, 0, 2, 3).reshape(128, 320),
        np.float32,
    )


def build_in_maps(refined_points, gt_points, visibility, projection_matrices,
                  tracks_2d, corr):
    beta, validc, colc, flat_idx = _host_band_tables()
    vis0 = visibility[:, 0, :] > 0                      # [4,1024]
    pair = (vis0[:, :, None] | vis0[:, colc]) & validc  # [4,1024,5]
    pairf = pair.astype(np.float64)
    g1_4 = 2.0 * beta * pairf                           # [4,1024,5]
    g2_4 = -(beta + 2.0 * beta**2) * pairf
    g1c = _to_dev_band(np.concatenate([g1_4, g1_4], 0))
    g2c = _to_dev_band(np.concatenate([g2_4, g2_4], 0))

    corr64 = corr.reshape(V * V * B, MATSZ)
    corr_bf = corr64.astype(ml_dtypes.bfloat16)
    pred_flat = refined_points.reshape(B * F_FRAMES * N, 3)
    gt_flat = gt_points.reshape(B * F_FRAMES * N, 3)
    vis_flat = visibility.reshape(B * F_FRAMES * N)
    pvals = projection_matrices.reshape(V * B, 12)

    in_maps = []
    for c in range(NCORES):
        mats = corr64[c * MAT_PER_CORE : (c + 1) * MAT_PER_CORE]
        band_c = _to_dev_band(mats[:, flat_idx])
        rp = pred_flat[c * REC_PTS : (c + 1) * REC_PTS].reshape(128, 192)
        rg = gt_flat[c * REC_PTS : (c + 1) * REC_PTS].reshape(128, 192)
        rv = vis_flat[c * REC_PTS : (c + 1) * REC_PTS].reshape(128, 64)
        vbs = [2 * c, 2 * c + 1]
        tks = np.stack([tracks_2d[vb // 4, vb % 4].reshape(128, 256) for vb in vbs])
        ipr = np.stack(
            [refined_points[vb % 4].reshape(128, 384) for vb in vbs]
        )
        pb = np.broadcast_to(
            np.concatenate([pvals[vb] for vb in vbs])[None, :], (128, 24)
        )
        in_maps.append({
            "corrm": np.ascontiguousarray(
                corr_bf[c * MAT_PER_CORE : (c + 1) * MAT_PER_CORE].reshape(-1)
            ),
            "bnd": band_c,
            "g1": g1c,
            "g2": g2c,
            "recpred": np.ascontiguousarray(rp),
            "recgt": np.ascontiguousarray(rg),
            "recvis": np.ascontiguousarray(rv),
            "trk": np.ascontiguousarray(tks, np.float32),
            "iprd": np.ascontiguousarray(ipr, np.float32),
            "projbc": np.ascontiguousarray(pb, np.float32),
        })
    return in_maps


def kernel(refined_points, gt_points, visibility, projection_matrices,
           tracks_2d, correspondence_matrices):
    global LAST_RESULTS
    refined_points = np.ascontiguousarray(refined_points, np.float32)
    gt_points = np.ascontiguousarray(gt_points, np.float32)
    visibility = np.ascontiguousarray(visibility, np.int32)
    projection_matrices = np.ascontiguousarray(projection_matrices, np.float32)
    tracks_2d = np.ascontiguousarray(tracks_2d, np.float32)
    corr = np.ascontiguousarray(correspondence_matrices, np.float32)

    if "nc" not in _CACHE:
        _CACHE["nc"] = _build_program()
    nc = _CACHE["nc"]

    in_maps = build_in_maps(
        refined_points, gt_points, visibility, projection_matrices,
        tracks_2d, corr,
    )

    trace = bool(int(os.environ.get("KERNEL_TRACE", "0")))
    ncr = int(os.environ.get("KERNEL_NCORES", str(NCORES)))
    res = run_bass_kernel_spmd(
        nc, in_maps[:ncr], core_ids=list(range(ncr)), trace=trace,
        stitch_traces=False,
    )
    LAST_RESULTS = res
    P = np.stack([r["out"] for r in res.results]).astype(np.float64)   # [8,128,24]

    # ---- attention ----
    att_sum = (
        P[:, :, C_ATT_MAIN].sum() + P[:, :, C_ATT_C1].sum() + P[:, :, C_ATT_C2].sum()
    )
    att = att_sum / (V * V * B * N * N)

    # ---- reconstruction ----
    se = P[:, :, C_REC_SE].sum()
    num = 3.0 * P[:, :, C_REC_NUM].sum()
    mn = P[:, :, C_REC_MN : C_REC_MN + 3].min(axis=(0, 1))
    mx = P[:, :, C_REC_MX : C_REC_MX + 3].max(axis=(0, 1))
    scale = (mx - mn).max() + 1e-6
    if not num > 0:
        scale = 1.0
    rec = (se / max(num, 1.0)) / scale**2

    # ---- identity ----
    vls = []
    for vb in range(V * B):
        c, i = vb // 2, vb % 2
        ex = P[c, :, C_ID_ERR + 2 * i]
        ey = P[c, :, C_ID_ERR + 2 * i + 1]
        mnx = P[c, :, C_ID_MN + 2 * i]
        mny = P[c, :, C_ID_MN + 2 * i + 1]
        mxx = P[c, :, C_ID_MX + 2 * i]
        mxy = P[c, :, C_ID_MX + 2 * i + 1]
        for f in range(F_FRAMES):
            s = slice(8 * f, 8 * f + 8)
            whx = max(224.0, mxx[s].max() - mnx[s].min() + 1e-6)
            why = max(224.0, mxy[s].max() - mny[s].min() + 1e-6)
            vls.append((ex[s].sum() / whx**2 + ey[s].sum() / why**2) / N)
    ident = float(np.mean(vls))

    total = 1.0 * rec + 1.0 * ident + 0.5 * att
    return (
        np.float32(total), np.float32(rec), np.float32(ident), np.float32(att),
    )


# revision 11
# speedup vs baseline: 1.3320x; 1.3320x over previous
"""Trainium2 Bass kernel for nn_CombinedLoss (rec + ident + attention-BCE).

Strategy
--------
The 256 MB correspondence_matrices BCE dominates (memory-bound).  Key identity:
gt_corr is nonzero only on the 5 diagonals |i-j|<=2 of each [N,N] matrix, so

    bce = w*(relu(x) - x*g + log1p(exp(-|x|)))   with w = 1+2g
        = softplus(x)                            off-band (~all elements)
        = softplus(x) + [2g*softplus(x) - (g+2g^2)*x]   on the band

Main stream (per core, 8 matrices, host-converted to bf16 = 16 MB DMA):
softplus sums are split across two engines so neither is the bottleneck:
  - K_SIG chunks: ScalarE Sigmoid pass, then DVE computes
    softplus = relu(x) + p(m), m = min(s, 1-s), p = deg-2 minimax poly of
    -ln(1-m) on [0, 0.5] (exact identity softplus = relu - ln(1-m)); the
    relu and poly sums come from fused accum_out ops (5 DVE ops/chunk).
  - remaining chunks: ScalarE Exp then Ln(bias=1) with accum_out; all Exps
    grouped before all Lns so each ACT table loads once (3 loads total).
Band correction: diagonals are extracted host-side into a dense [128,320]
f32 tile; pair visibility is folded into host-precomputed weights.

rec + ident losses are tiny (~3.5 MB) and sharded uniformly: each core takes
1/8 of the reconstruction points and 2 of the 16 (view,batch) identity pairs.
Each core writes a [128,24] f32 partial; the host only combines partials.
"""

import dataclasses
import os

import ml_dtypes
import numpy as np

import concourse.bacc as bacc
import concourse.bass as bass
import concourse.mybir as mybir
from concourse.bass_utils import run_bass_kernel_spmd
from concourse.tile import TileContext

F32 = mybir.dt.float32
BF16 = mybir.dt.bfloat16
I32 = mybir.dt.int32
AF = mybir.ActivationFunctionType
OP = mybir.AluOpType
AX = mybir.AxisListType

N = 1024
V = 4
B = 4
F_FRAMES = 16
NCORES = 8
MAT_PER_CORE = 8          # V*V*B / 8
MATSZ = N * N             # elements per matrix
NCHUNKS = 16
CHUNK = MAT_PER_CORE * MATSZ // NCHUNKS      # 524288 elems
CHUNK_F = CHUNK // 128                       # free dim per partition

K_SIG = int(os.environ.get("KERNEL_KSIG", "9"))

# deg-2 minimax fit of -ln(1-m) ~ m*(PC1 + PC2*m) on m in [0, 0.5]
PC1 = 0.92217051
PC2 = 0.91185078

# rec shard: 1/8 of B*F*N = 65536 points -> 8192 points = [128, 64, 3]
REC_PTS = 8192

# final_acc column layout
C_ATT_MAIN = 0
C_ATT_C1 = 1
C_REC_SE = 2
C_REC_NUM = 3
C_REC_MN = 4     # 4..6
C_REC_MX = 7     # 7..9
C_ID_ERR = 10    # 10..13 (vb0x, vb0y, vb1x, vb1y)
C_ID_MN = 14     # 14..17
C_ID_MX = 18     # 18..21
C_ATT_C2 = 22
C_ATT_PS = 23    # psum matmul-accumulated sums (x128, host divides)
NCOLS = 24

NACC = 32        # acc cols: 0..15 chunk (ln or poly) accums, 16..31 relu accums

_CACHE = {}
LAST_RESULTS = None


def _ap(t, offset, pairs):
    """Custom access pattern on a DRAM tensor handle."""
    return dataclasses.replace(t[:], ap=[list(p) for p in pairs], offset=offset)


def _build_program():
    parts = set(
        os.environ.get("KERNEL_PARTS", "main,band,rec,ident").split(",")
    )
    nc = bacc.Bacc("TRN2", target_bir_lowering=False, debug=False)

    corrm = nc.dram_tensor(
        "corrm", [MAT_PER_CORE * MATSZ], BF16, kind="ExternalInput"
    )
    bnd_d = nc.dram_tensor("bnd", [128, 320], F32, kind="ExternalInput")
    g1_d = nc.dram_tensor("g1", [128, 320], F32, kind="ExternalInput")
    g2_d = nc.dram_tensor("g2", [128, 320], F32, kind="ExternalInput")
    recpred = nc.dram_tensor("recpred", [128, 192], F32, kind="ExternalInput")
    recgt = nc.dram_tensor("recgt", [128, 192], F32, kind="ExternalInput")
    recvis = nc.dram_tensor("recvis", [128, 64], I32, kind="ExternalInput")
    trk = nc.dram_tensor("trk", [2, 128, 256], F32, kind="ExternalInput")
    iprd = nc.dram_tensor("iprd", [2, 128, 384], F32, kind="ExternalInput")
    projbc = nc.dram_tensor("projbc", [128, 24], F32, kind="ExternalInput")
    out_d = nc.dram_tensor("out", [128, NCOLS], F32, kind="ExternalOutput")

    with TileContext(nc) as tc:
        with (
            tc.tile_pool(name="xpool", bufs=3) as xpool,
            tc.tile_pool(name="spool", bufs=2) as spool,
            tc.tile_pool(name="wpool", bufs=1) as wpool,
            tc.tile_pool(name="mmpool", bufs=2) as mmpool,
            tc.tile_pool(name="cpool", bufs=1) as cpool,
            tc.psum_pool(name="ppool", bufs=1) as ppool,
        ):
            acc = cpool.tile([128, NACC], F32, tag="acc")
            fin = cpool.tile([128, NCOLS], F32, tag="fin")
            nc.vector.memset(acc[:], 0.0)
            nc.vector.memset(fin[:], 0.0)

            if "band" in parts:
                bt = cpool.tile([128, 320], F32, tag="bt")
                g1t = cpool.tile([128, 320], F32, tag="g1t")
                g2t = cpool.tile([128, 320], F32, tag="g2t")
                nc.gpsimd.dma_start(bt[:], bnd_d[:])
                nc.gpsimd.dma_start(g1t[:], g1_d[:])
                nc.gpsimd.dma_start(g2t[:], g2_d[:])

            # ---------------- main BCE stream ----------------
            # phase S: sigmoid + DVE polynomial chunks; per-chunk sums are
            # done by the (otherwise idle) TensorEngine: ones^T @ tile
            # accumulated into one PSUM bank across all chunks.
            NMM = CHUNK_F // 512
            n_mm_total = K_SIG * 2 * NMM
            mm_idx = 0
            if "main" in parts and K_SIG > 0:
                ones = cpool.tile([128, 128], BF16, tag="ones")
                nc.vector.memset(ones[:], 1.0)
                ps = ppool.tile([128, 512], F32, tag="ps")
                for k in range(K_SIG):
                    xt = xpool.tile([128, CHUNK_F], BF16, tag="xt")
                    nc.sync.dma_start(
                        xt[:],
                        _ap(corrm, k * CHUNK, [[CHUNK_F, 128], [1, CHUNK_F]]),
                    )
                    st = spool.tile([128, CHUNK_F], BF16, tag="st")
                    nc.scalar.activation(st[:], xt[:], AF.Sigmoid)
                    # relu(x)
                    rt = mmpool.tile([128, CHUNK_F], BF16, tag="rt")
                    nc.vector.tensor_scalar(rt[:], xt[:], 0.0, None, OP.max)
                    # m = min(s, 1-s)
                    nst = wpool.tile([128, CHUNK_F], BF16, tag="nst")
                    nc.vector.tensor_scalar(
                        nst[:], st[:], -1.0, 1.0, OP.mult, OP.add
                    )
                    mt = wpool.tile([128, CHUNK_F], BF16, tag="mt")
                    nc.vector.tensor_tensor(mt[:], st[:], nst[:], OP.min)
                    # p(m) = m*(PC1 + PC2*m)
                    yt = wpool.tile([128, CHUNK_F], BF16, tag="yt")
                    nc.vector.tensor_scalar(
                        yt[:], mt[:], PC2, PC1, OP.mult, OP.add
                    )
                    wt = mmpool.tile([128, CHUNK_F], BF16, tag="wt")
                    nc.vector.tensor_tensor(wt[:], yt[:], mt[:], OP.mult)
                    for tile in (rt, wt):
                        for j in range(NMM):
                            nc.tensor.matmul(
                                ps[:],
                                ones[:],
                                tile[:, j * 512 : (j + 1) * 512],
                                start=(mm_idx == 0),
                                stop=(mm_idx == n_mm_total - 1),
                            )
                            mm_idx += 1

                # phase E: exp for the remaining chunks (one table load)
                ets = {}
                for k in range(K_SIG, NCHUNKS):
                    xt = xpool.tile([128, CHUNK_F], BF16, tag="xt")
                    nc.sync.dma_start(
                        xt[:],
                        _ap(corrm, k * CHUNK, [[CHUNK_F, 128], [1, CHUNK_F]]),
                    )
                    et = cpool.tile([128, CHUNK_F], BF16, tag=f"et{k}")
                    nc.scalar.activation(et[:], xt[:], AF.Exp)
                    ets[k] = et

            if "band" in parts:
                eb = cpool.tile([128, 320], F32, tag="eb")
                nc.scalar.activation(eb[:], bt[:], AF.Exp)

            # phase L: ln(1+e^x) with fused accum (one table load)
            if "main" in parts:
                for k in range(K_SIG, NCHUNKS):
                    ot = spool.tile([128, CHUNK_F], BF16, tag="ot")
                    nc.scalar.activation(
                        ot[:], ets[k][:], AF.Ln, bias=1.0,
                        accum_out=acc[:, k : k + 1],
                    )

            if "band" in parts:
                spb = cpool.tile([128, 320], F32, tag="spb")
                nc.scalar.activation(spb[:], eb[:], AF.Ln, bias=1.0)
                scr1 = cpool.tile([128, 320], F32, tag="scr1")
                scr2 = cpool.tile([128, 320], F32, tag="scr2")
                nc.vector.tensor_tensor(scr1[:], g1t[:], spb[:], OP.mult)
                nc.vector.tensor_reduce(
                    fin[:, C_ATT_C1 : C_ATT_C1 + 1], scr1[:], axis=AX.X, op=OP.add
                )
                nc.vector.tensor_tensor(scr2[:], g2t[:], bt[:], OP.mult)
                nc.vector.tensor_reduce(
                    fin[:, C_ATT_C2 : C_ATT_C2 + 1], scr2[:], axis=AX.X, op=OP.add
                )

            # ---------------- reconstruction loss partials ----------------
            if "rec" in parts:
                prt = cpool.tile([128, 192], F32, tag="prt")
                grt = cpool.tile([128, 192], F32, tag="grt")
                vrt = cpool.tile([128, 64], I32, tag="vrt")
                nc.gpsimd.dma_start(prt[:], recpred[:])
                nc.gpsimd.dma_start(grt[:], recgt[:])
                nc.gpsimd.dma_start(vrt[:], recvis[:])
                mf = cpool.tile([128, 64], F32, tag="mf")
                nc.vector.tensor_copy(mf[:], vrt[:])
                dd = cpool.tile([128, 192], F32, tag="dd")
                nc.vector.tensor_tensor(dd[:], prt[:], grt[:], OP.subtract)
                d2 = cpool.tile([128, 192], F32, tag="d2")
                nc.vector.tensor_tensor(d2[:], dd[:], dd[:], OP.mult)
                se3 = cpool.tile([128, 64], F32, tag="se3")
                nc.vector.tensor_reduce(
                    se3[:], d2[:].rearrange("p (q c) -> p q c", c=3), axis=AX.X, op=OP.add
                )
                se3m = cpool.tile([128, 64], F32, tag="se3m")
                nc.vector.tensor_tensor(se3m[:], se3[:], mf[:], OP.mult)
                nc.vector.tensor_reduce(
                    fin[:, C_REC_SE : C_REC_SE + 1], se3m[:], axis=AX.X, op=OP.add
                )
                nc.vector.tensor_reduce(
                    fin[:, C_REC_NUM : C_REC_NUM + 1], mf[:], axis=AX.X, op=OP.add
                )
                # masked min / max of gt per coordinate
                bmn = cpool.tile([128, 192], F32, tag="bmn")
                bmx = cpool.tile([128, 192], F32, tag="bmx")
                nc.vector.memset(bmn[:], 1e30)
                nc.vector.memset(bmx[:], -1e30)
                for c in range(3):
                    nc.vector.copy_predicated(
                        bmn[:, c : 192 : 3], vrt[:], grt[:, c : 192 : 3]
                    )
                    nc.vector.copy_predicated(
                        bmx[:, c : 192 : 3], vrt[:], grt[:, c : 192 : 3]
                    )
                nc.vector.tensor_reduce(
                    fin[:, C_REC_MN : C_REC_MN + 3],
                    bmn[:].rearrange("p (q c) -> p c q", c=3), axis=AX.X, op=OP.min,
                )
                nc.vector.tensor_reduce(
                    fin[:, C_REC_MX : C_REC_MX + 3],
                    bmx[:].rearrange("p (q c) -> p c q", c=3), axis=AX.X, op=OP.max,
                )

            # ---------------- identity loss partials (2 vb slots) ----------------
            if "ident" in parts:
                psb = cpool.tile([128, 24], F32, tag="psb")
                nc.gpsimd.dma_start(psb[:], projbc[:])

                for i in range(2):
                    tk = cpool.tile([128, 256], F32, tag=f"tk{i}")
                    pd = cpool.tile([128, 384], F32, tag=f"pd{i}")
                    nc.gpsimd.dma_start(tk[:], trk[i])
                    nc.gpsimd.dma_start(pd[:], iprd[i])
                    Xc = pd[:, 0:384:3]
                    Yc = pd[:, 1:384:3]
                    Zc = pd[:, 2:384:3]

                    def cS(col):
                        return psb[:, col : col + 1]

                    base = i * 12

                    def lincomb(row, tag):
                        # P[row,0]*x + P[row,1]*y + P[row,2]*z + P[row,3]
                        t0 = cpool.tile([128, 128], F32, tag=f"{tag}0_{i}")
                        t1 = cpool.tile([128, 128], F32, tag=f"{tag}1_{i}")
                        nc.vector.tensor_scalar(
                            t0[:], Xc, cS(base + row * 4 + 0), cS(base + row * 4 + 3),
                            OP.mult, OP.add,
                        )
                        nc.vector.tensor_scalar(
                            t1[:], Yc, cS(base + row * 4 + 1), None, OP.mult
                        )
                        nc.vector.tensor_tensor(t0[:], t0[:], t1[:], OP.add)
                        nc.vector.tensor_scalar(
                            t1[:], Zc, cS(base + row * 4 + 2), None, OP.mult
                        )
                        nc.vector.tensor_tensor(t0[:], t0[:], t1[:], OP.add)
                        return t0

                    den = lincomb(2, "den")
                    nc.vector.tensor_scalar_add(den[:], den[:], 1e-10)
                    rd = cpool.tile([128, 128], F32, tag=f"rd{i}")
                    nc.vector.reciprocal(rd[:], den[:])
                    nx = lincomb(0, "nx")
                    ny = lincomb(1, "ny")
                    nc.vector.tensor_tensor(nx[:], nx[:], rd[:], OP.mult)
                    nc.vector.tensor_tensor(ny[:], ny[:], rd[:], OP.mult)
                    nc.vector.tensor_tensor(nx[:], nx[:], tk[:, 0:256:2], OP.subtract)
                    nc.vector.tensor_tensor(ny[:], ny[:], tk[:, 1:256:2], OP.subtract)
                    sqx = cpool.tile([128, 128], F32, tag=f"sqx{i}")
                    nc.vector.tensor_tensor(sqx[:], nx[:], nx[:], OP.mult)
                    nc.vector.tensor_reduce(
                        fin[:, C_ID_ERR + 2 * i : C_ID_ERR + 2 * i + 1],
                        sqx[:], axis=AX.X, op=OP.add,
                    )
                    sqy = cpool.tile([128, 128], F32, tag=f"sqy{i}")
                    nc.vector.tensor_tensor(sqy[:], ny[:], ny[:], OP.mult)
                    nc.vector.tensor_reduce(
                        fin[:, C_ID_ERR + 2 * i + 1 : C_ID_ERR + 2 * i + 2],
                        sqy[:], axis=AX.X, op=OP.add,
                    )
                    tkv = tk[:].rearrange("p (q c) -> p c q", c=2)
                    nc.vector.tensor_reduce(
                        fin[:, C_ID_MN + 2 * i : C_ID_MN + 2 * i + 2],
                        tkv, axis=AX.X, op=OP.min,
                    )
                    nc.vector.tensor_reduce(
                        fin[:, C_ID_MX + 2 * i : C_ID_MX + 2 * i + 2],
                        tkv, axis=AX.X, op=OP.max,
                    )

            # ---------------- final reductions + store ----------------
            if "main" in parts and K_SIG > 0:
                # every psum lane holds the same grand total; host /128
                nc.vector.tensor_reduce(
                    fin[:, C_ATT_PS : C_ATT_PS + 1], ps[:], axis=AX.X, op=OP.add
                )
            nc.vector.tensor_reduce(
                fin[:, C_ATT_MAIN : C_ATT_MAIN + 1], acc[:], axis=AX.X, op=OP.add
            )
            nc.sync.dma_start(out_d[:], fin[:])

    nc.compile()
    return nc


def _host_band_tables():
    """Banded weight tables with pair visibility folded in (host-side)."""
    beta = np.array([0.49, 0.7, 1.0, 0.7, 0.49], np.float64)
    r = np.arange(N)
    d_off = np.arange(5) - 2
    col = r[:, None] + d_off[None, :]                  # [1024,5]
    validc = (col >= 0) & (col < N)
    colc = np.clip(col, 0, N - 1)
    # flat index of band element (r, r+d) within one matrix, clipped
    flat_idx = np.clip(r[:, None] * 1025 + np.arange(5)[None, :] - 2, 0, MATSZ - 1)
    return beta, validc, colc, flat_idx


def _to_dev_band(a):
    """[8,1024,5] -> [128, 320] with partition p holding rows 8p..8p+7."""
    return np.ascontiguousarray(
        a.reshape(8, 128, 8, 5).transpose(1, 0, 2, 3).reshape(128, 320),
        np.float32,
    )


def build_in_maps(refined_points, gt_points, visibility, projection_matrices,
                  tracks_2d, corr):
    beta, validc, colc, flat_idx = _host_band_tables()
    vis0 = visibility[:, 0, :] > 0                      # [4,1024]
    pair = (vis0[:, :, None] | vis0[:, colc]) & validc  # [4,1024,5]
    pairf = pair.astype(np.float64)
    g1_4 = 2.0 * beta * pairf                           # [4,1024,5]
    g2_4 = -(beta + 2.0 * beta**2) * pairf
    g1c = _to_dev_band(np.concatenate([g1_4, g1_4], 0))
    g2c = _to_dev_band(np.concatenate([g2_4, g2_4], 0))

    corr64 = corr.reshape(V * V * B, MATSZ)
    corr_bf = corr64.astype(ml_dtypes.bfloat16)
    pred_flat = refined_points.reshape(B * F_FRAMES * N, 3)
    gt_flat = gt_points.reshape(B * F_FRAMES * N, 3)
    vis_flat = visibility.reshape(B * F_FRAMES * N)
    pvals = projection_matrices.reshape(V * B, 12)

    in_maps = []
    for c in range(NCORES):
        mats = corr64[c * MAT_PER_CORE : (c + 1) * MAT_PER_CORE]
        band_c = _to_dev_band(mats[:, flat_idx])
        rp = pred_flat[c * REC_PTS : (c + 1) * REC_PTS].reshape(128, 192)
        rg = gt_flat[c * REC_PTS : (c + 1) * REC_PTS].reshape(128, 192)
        rv = vis_flat[c * REC_PTS : (c + 1) * REC_PTS].reshape(128, 64)
        vbs = [2 * c, 2 * c + 1]
        tks = np.stack([tracks_2d[vb // 4, vb % 4].reshape(128, 256) for vb in vbs])
        ipr = np.stack(
            [refined_points[vb % 4].reshape(128, 384) for vb in vbs]
        )
        pb = np.broadcast_to(
            np.concatenate([pvals[vb] for vb in vbs])[None, :], (128, 24)
        )
        in_maps.append({
            "corrm": np.ascontiguousarray(
                corr_bf[c * MAT_PER_CORE : (c + 1) * MAT_PER_CORE].reshape(-1)
            ),
            "bnd": band_c,
            "g1": g1c,
            "g2": g2c,
            "recpred": np.ascontiguousarray(rp),
            "recgt": np.ascontiguousarray(rg),
            "recvis": np.ascontiguousarray(rv),
            "trk": np.ascontiguousarray(tks, np.float32),
            "iprd": np.ascontiguousarray(ipr, np.float32),
            "projbc": np.ascontiguousarray(pb, np.float32),
        })
    return in_maps


def kernel(refined_points, gt_points, visibility, projection_matrices,
           tracks_2d, correspondence_matrices):
    global LAST_RESULTS
    refined_points = np.ascontiguousarray(refined_points, np.float32)
    gt_points = np.ascontiguousarray(gt_points, np.float32)
    visibility = np.ascontiguousarray(visibility, np.int32)
    projection_matrices = np.ascontiguousarray(projection_matrices, np.float32)
    tracks_2d = np.ascontiguousarray(tracks_2d, np.float32)
    corr = np.ascontiguousarray(correspondence_matrices, np.float32)

    if "nc" not in _CACHE:
        _CACHE["nc"] = _build_program()
    nc = _CACHE["nc"]

    in_maps = build_in_maps(
        refined_points, gt_points, visibility, projection_matrices,
        tracks_2d, corr,
    )

    trace = bool(int(os.environ.get("KERNEL_TRACE", "0")))
    ncr = int(os.environ.get("KERNEL_NCORES", str(NCORES)))
    res = run_bass_kernel_spmd(
        nc, in_maps[:ncr], core_ids=list(range(ncr)), trace=trace,
        stitch_traces=False,
    )
    LAST_RESULTS = res
    P = np.stack([r["out"] for r in res.results]).astype(np.float64)   # [8,128,24]

    # ---- attention ----
    att_sum = (
        P[:, :, C_ATT_MAIN].sum() + P[:, :, C_ATT_C1].sum() + P[:, :, C_ATT_C2].sum()
        + P[:, :, C_ATT_PS].sum() / 128.0
    )
    att = att_sum / (V * V * B * N * N)

    # ---- reconstruction ----
    se = P[:, :, C_REC_SE].sum()
    num = 3.0 * P[:, :, C_REC_NUM].sum()
    mn = P[:, :, C_REC_MN : C_REC_MN + 3].min(axis=(0, 1))
    mx = P[:, :, C_REC_MX : C_REC_MX + 3].max(axis=(0, 1))
    scale = (mx - mn).max() + 1e-6
    if not num > 0:
        scale = 1.0
    rec = (se / max(num, 1.0)) / scale**2

    # ---- identity ----
    vls = []
    for vb in range(V * B):
        c, i = vb // 2, vb % 2
        ex = P[c, :, C_ID_ERR + 2 * i]
        ey = P[c, :, C_ID_ERR + 2 * i + 1]
        mnx = P[c, :, C_ID_MN + 2 * i]
        mny = P[c, :, C_ID_MN + 2 * i + 1]
        mxx = P[c, :, C_ID_MX + 2 * i]
        mxy = P[c, :, C_ID_MX + 2 * i + 1]
        for f in range(F_FRAMES):
            s = slice(8 * f, 8 * f + 8)
            whx = max(224.0, mxx[s].max() - mnx[s].min() + 1e-6)
            why = max(224.0, mxy[s].max() - mny[s].min() + 1e-6)
            vls.append((ex[s].sum() / whx**2 + ey[s].sum() / why**2) / N)
    ident = float(np.mean(vls))

    total = 1.0 * rec + 1.0 * ident + 0.5 * att
    return (
        np.float32(total), np.float32(rec), np.float32(ident), np.float32(att),
    )
